# revision 16
# baseline (speedup 1.0000x reference)
"""AttentionDot kernel for Trainium2 (Bass/Tile), 8-core data-parallel over batch.

Math (per batch b):
    prod[tp,tq,d] = q[tq,d] * p[tp,d]
    scores[tp,tq] = tanh(prod @ W) @ vd
    weights       = softmax(scores over tp)
    out[tp,d]     = sum_tq weights[tp,tq] * q[tq,d]

Per-core layout strategy:
    - pT [d,tp], qT [d,tq] transposed once via PE-transpose.
    - For each tq pair (2i, 2i+1): prodT [d, 512] = pT * qT[:,tq] (DVE per-partition
      scalar mul, bf16 out), then S^T = W^T @ prodT via one matmul (W stationary).
    - tanh on ScalarE in 3-bank chunks, PSUM -> SBUF bf16.
    - vd-reduction via "sliding delta" matmuls: lhsT = [128,128] window of a
      [128,255] buffer that is all zeros except vd at col 127; window for pair i
      puts vd in column i, so out row i accumulates [vd.T @ T_2i | vd.T @ T_2i+1].
      128 such matmuls accumulate the whole scores^T [tq-packed, tp] into ONE
      PSUM bank - no [1,N] PSUM rows to evacuate.
    - exp with fused free-axis accum (Z per tq), qn = q/Z, final matmul
      out[tp,d] = sum_i E[i,tp]*qn[i,d] (even + odd accumulated).
"""

import numpy as np

B, TQ, TP, D = 8, 256, 256, 128
NCORES = 8
NPAIR = TQ // 2  # 128 pairs of tq
GP = 3  # pairs per PSUM staging group (3 banks of 2KB; tanh chunks of [128, 1536])

_nc_cache = {}


def _build_nc(repeat=1):
    from contextlib import ExitStack

    import concourse.bacc as bacc
    import concourse.tile as tile
    from concourse import mybir
    from concourse.masks import make_identity

    f32 = mybir.dt.float32
    bf16 = mybir.dt.bfloat16
    AF = mybir.ActivationFunctionType

    nc = bacc.Bacc("TRN2", target_bir_lowering=False, debug=False)
    q_d = nc.dram_tensor("q", [TQ, D], f32, kind="ExternalInput").ap()
    p_d = nc.dram_tensor("p", [TP, D], f32, kind="ExternalInput").ap()
    w_d = nc.dram_tensor("W", [D, D], f32, kind="ExternalInput").ap()
    vd_d = nc.dram_tensor("vd", [D, 1], f32, kind="ExternalInput").ap()
    out_d = nc.dram_tensor("out", [TP, D], f32, kind="ExternalOutput").ap()

    with tile.TileContext(nc) as tc, ExitStack() as ctx:
        consts = ctx.enter_context(tc.tile_pool(name="consts", bufs=1))
        nat_pool = ctx.enter_context(tc.tile_pool(name="nat", bufs=2))
        prod_pool = ctx.enter_context(tc.tile_pool(name="prod", bufs=3))
        t_pool = ctx.enter_context(tc.tile_pool(name="tsb", bufs=3))
        s_pool = ctx.enter_context(tc.tile_pool(name="sps", bufs=2, space="PSUM"))
        sc_pool = ctx.enter_context(tc.tile_pool(name="scps", bufs=1, space="PSUM"))

        # ---------------- constants / setup ----------------
        ident = consts.tile([128, 128], f32, name="ident", tag="ident")
        make_identity(nc, ident)

        w_f = consts.tile([D, D], f32, name="w_f", tag="w_f")
        nc.sync.dma_start(w_f, w_d)
        w_bf = consts.tile([D, D], bf16, name="w_bf", tag="w_bf")
        nc.vector.tensor_copy(w_bf, w_f)

        vd_f = consts.tile([D, 1], f32, name="vd_f", tag="vd_f")
        nc.sync.dma_start(vd_f, vd_d)
        # sliding-window delta weights: zeros with vd at column NPAIR-1
        vdw = consts.tile([D, 2 * NPAIR - 1], bf16, name="vdw", tag="vdw")
        nc.vector.memset(vdw, 0.0)
        nc.vector.tensor_copy(vdw[:, NPAIR - 1 : NPAIR], vd_f)

        # q rows interleaved even/odd: partition t holds [q[2t,:] | q[2t+1,:]]
        q_eo = consts.tile([NPAIR, 2 * D], f32, name="q_eo", tag="q_eo")
        nc.sync.dma_start(q_eo, q_d.rearrange("(t two) d -> t (two d)", two=2))

        # transposes: qT [d, tq] (f32, used as per-partition scalars), pT bf16
        qT = consts.tile([D, TQ], f32, name="qT", tag="qT")
        pT = consts.tile([D, TP], bf16, name="pT", tag="pT")
        for src_d, dstT, nm in ((q_d, qT, "q"), (p_d, pT, "p")):
            tr_ps = s_pool.tile([128, GP * 512], f32, name=f"trps_{nm}", tag="s")
            for h in range(2):
                nat = nat_pool.tile([128, D], f32, name=f"nat_{nm}{h}", tag="nat")
                nc.sync.dma_start(nat, src_d[h * 128 : (h + 1) * 128, :])
                nc.tensor.transpose(tr_ps[:, h * 128 : (h + 1) * 128], nat, ident)
            nc.vector.tensor_copy(dstT[:, 0:128], tr_ps[:, 0:128])
            nc.vector.tensor_copy(dstT[:, 128:256], tr_ps[:, 128:256])

        zeros_sb = consts.tile([128, 2 * TP], bf16, name="zeros_sb", tag="zeros")
        nc.vector.memset(zeros_sb, 0.0)

        # ---------------- main compute body ----------------
        # scores^T accumulator: row i = [scores[:,2i].T | scores[:,2i+1].T].
        # Delta matmuls are col-tiled: pair i -> PE col-group i//32, so rows
        # 32g..32g+31 are written by 32-column matmuls at tile_position (0,32g).
        # Pre-clear the bank with 4 zero-matmuls (start=True, stop=False) so
        # every delta accumulates with start=False (no mid-stream bank clears).
        def body():
            sc_ps = sc_pool.tile([128, 2 * TP], f32, name="sc_ps", tag="sc")
            for g in range(4):
                nc.tensor.matmul(
                    sc_ps[32 * g : 32 * (g + 1), :],
                    lhsT=zeros_sb[:, 0:32],
                    rhs=zeros_sb,
                    start=True,
                    stop=False,
                    skip_group_check=True,
                    tile_position=(0, 32 * g),
                )

            # process pairs in col-group-interleaved order so consecutive delta
            # matmuls target different PE column groups (they run concurrently)
            order = [32 * g + c for c in range(32) for g in range(4)]
            groups = [order[k : k + GP] for k in range(0, len(order), GP)]

            for gi, pairs in enumerate(groups):
                n = len(pairs)
                s_ps = s_pool.tile([128, GP * 512], f32, name=f"s_ps_{gi}", tag="s")
                t_sb = t_pool.tile([128, GP * 512], bf16, name=f"t_sb_{gi}", tag="t")
                for k, i in enumerate(pairs):
                    prod = prod_pool.tile(
                        [128, 512], bf16, name=f"prod_{i}", tag="prod"
                    )
                    nc.vector.tensor_scalar_mul(
                        prod[:, 0:TP], pT, qT[:, 2 * i : 2 * i + 1]
                    )
                    nc.vector.tensor_scalar_mul(
                        prod[:, TP : 2 * TP], pT, qT[:, 2 * i + 1 : 2 * i + 2]
                    )
                    nc.tensor.matmul(
                        s_ps[:, k * 512 : (k + 1) * 512],
                        lhsT=w_bf,
                        rhs=prod,
                        start=True,
                        stop=True,
                    )
                nc.scalar.activation(
                    t_sb[:, : n * 512], s_ps[:, : n * 512], AF.Tanh
                )
                for k, i in enumerate(pairs):
                    g, c = i // 32, i % 32
                    nc.tensor.matmul(
                        sc_ps[32 * g : 32 * (g + 1), :],
                        lhsT=vdw[:, NPAIR - 1 - c : NPAIR - 1 - c + 32],
                        rhs=t_sb[:, k * 512 : (k + 1) * 512],
                        start=False,
                        stop=(c == 31),
                        skip_group_check=True,
                        tile_position=(0, 32 * g),
                    )

            # ---------- softmax (over tp, the free axis) + output ----------
            e_sb = consts.tile([128, 2 * TP], f32, name="e_sb", tag="e_sb")
            z = consts.tile([128, 2], f32, name="z", tag="z")
            nc.scalar.activation(
                e_sb[:, 0:TP], sc_ps[:, 0:TP], AF.Exp, accum_out=z[:, 0:1]
            )
            nc.scalar.activation(
                e_sb[:, TP : 2 * TP], sc_ps[:, TP : 2 * TP], AF.Exp, accum_out=z[:, 1:2]
            )
            rz = consts.tile([128, 2], f32, name="rz", tag="rz")
            nc.vector.reciprocal(rz, z)
            qn = consts.tile([128, 2 * D], f32, name="qn", tag="qn")
            nc.vector.tensor_scalar_mul(qn[:, 0:D], q_eo[:, 0:D], rz[:, 0:1])
            nc.vector.tensor_scalar_mul(
                qn[:, D : 2 * D], q_eo[:, D : 2 * D], rz[:, 1:2]
            )

            # out[tp,d] = sum_i E_even[i,tp]*qn_even[i,d] + E_odd[i,tp]*qn_odd[i,d]
            out_ps = sc_pool.tile([128, TP], f32, name="out_ps", tag="sc")
            for c in range(2):
                nc.tensor.matmul(
                    out_ps[:, c * D : (c + 1) * D],
                    lhsT=e_sb[:, c * 128 : (c + 1) * 128],
                    rhs=qn[:, 0:D],
                    start=True,
                    stop=False,
                )
                nc.tensor.matmul(
                    out_ps[:, c * D : (c + 1) * D],
                    lhsT=e_sb[:, TP + c * 128 : TP + (c + 1) * 128],
                    rhs=qn[:, D : 2 * D],
                    start=False,
                    stop=True,
                )
            out_sb = consts.tile([128, TP], f32, name="out_sb", tag="out_sb")
            nc.vector.tensor_copy(out_sb, out_ps)
            nc.sync.dma_start(out_d[0:128, :], out_sb[:, 0:128])
            nc.sync.dma_start(out_d[128:256, :], out_sb[:, 128:256])

        if repeat == 1:
            body()
        else:
            # timing-only path: body > 256 instructions/engine, so arm the
            # back-edge branch prefetch to keep the loop overhead small
            with tc.For_i(
                0,
                repeat,
                1,
                hint_engines=(
                    mybir.EngineType.PE,
                    mybir.EngineType.DVE,
                    mybir.EngineType.Activation,
                ),
            ):
                body()

    nc.compile()
    return nc


def get_nc(repeat=1):
    key = ("nc", repeat)
    if key not in _nc_cache:
        _nc_cache[key] = _build_nc(repeat)
    return _nc_cache[key]


last_results = None


def kernel(q, p, W, vd, _repeat=1):
    global last_results
    from concourse.bass_utils import run_bass_kernel_spmd

    q = np.ascontiguousarray(np.asarray(q), dtype=np.float32)
    p = np.ascontiguousarray(np.asarray(p), dtype=np.float32)
    W = np.ascontiguousarray(np.asarray(W), dtype=np.float32)
    vd = np.ascontiguousarray(np.asarray(vd), dtype=np.float32)

    nc = get_nc(_repeat)
    in_maps = [
        {"q": q[b], "p": p[b], "W": W, "vd": vd} for b in range(B)
    ]
    res = run_bass_kernel_spmd(nc, in_maps, core_ids=list(range(NCORES)))
    last_results = res
    return np.stack([r["out"] for r in res.results], axis=0)


# revision 22
# speedup vs baseline: 1.1546x; 1.1546x over previous
"""AttentionDot kernel for Trainium2 (Bass/Tile), 8-core data-parallel over batch.

Math (per batch b):
    prod[tp,tq,d] = q[tq,d] * p[tp,d]
    scores[tp,tq] = tanh(prod @ W) @ vd
    weights       = softmax(scores over tp)
    out[tp,d]     = sum_tq weights[tp,tq] * q[tq,d]

Per-core layout strategy:
    - pT [d,tp], qT [d,tq] transposed once via PE-transpose.
    - For each tq pair (2i, 2i+1): prodT [d, 512] = pT * qT[:,tq] (DVE per-partition
      scalar mul, bf16 out), then S^T = W^T @ prodT via one matmul (W stationary).
    - tanh on ScalarE in 3-bank chunks, PSUM -> SBUF bf16.
    - vd-reduction via "sliding delta" matmuls: lhsT = [128,128] window of a
      [128,255] buffer that is all zeros except vd at col 127; window for pair i
      puts vd in column i, so out row i accumulates [vd.T @ T_2i | vd.T @ T_2i+1].
      128 such matmuls accumulate the whole scores^T [tq-packed, tp] into ONE
      PSUM bank - no [1,N] PSUM rows to evacuate.
    - exp with fused free-axis accum (Z per tq), qn = q/Z, final matmul
      out[tp,d] = sum_i E[i,tp]*qn[i,d] (even + odd accumulated).
"""

import numpy as np

B, TQ, TP, D = 8, 256, 256, 128
NCORES = 8
NPAIR = TQ // 2  # 128 pairs of tq
GP = 3  # pairs per PSUM staging group (3 banks of 2KB; tanh chunks of [128, 1536])

_nc_cache = {}


def _build_nc(repeat=1, gp=GP, prod_bufs=3, t_bufs=3, s_bufs=2, delta_mode="col"):
    from contextlib import ExitStack

    import concourse.bacc as bacc
    import concourse.tile as tile
    from concourse import mybir
    from concourse.masks import make_identity

    f32 = mybir.dt.float32
    bf16 = mybir.dt.bfloat16
    AF = mybir.ActivationFunctionType

    nc = bacc.Bacc("TRN2", target_bir_lowering=False, debug=False)
    q_d = nc.dram_tensor("q", [TQ, D], f32, kind="ExternalInput").ap()
    p_d = nc.dram_tensor("p", [TP, D], f32, kind="ExternalInput").ap()
    w_d = nc.dram_tensor("W", [D, D], f32, kind="ExternalInput").ap()
    vd_d = nc.dram_tensor("vd", [D, 1], f32, kind="ExternalInput").ap()
    out_d = nc.dram_tensor("out", [TP, D], f32, kind="ExternalOutput").ap()

    with tile.TileContext(nc) as tc, ExitStack() as ctx:
        consts = ctx.enter_context(tc.tile_pool(name="consts", bufs=1))
        nat_pool = ctx.enter_context(tc.tile_pool(name="nat", bufs=2))
        prod_pool = ctx.enter_context(tc.tile_pool(name="prod", bufs=prod_bufs))
        t_pool = ctx.enter_context(tc.tile_pool(name="tsb", bufs=t_bufs))
        s_pool = ctx.enter_context(tc.tile_pool(name="sps", bufs=s_bufs, space="PSUM"))
        sc_pool = ctx.enter_context(tc.tile_pool(name="scps", bufs=1, space="PSUM"))

        # ---------------- constants / setup ----------------
        ident = consts.tile([128, 128], f32, name="ident", tag="ident")
        make_identity(nc, ident)

        w_f = consts.tile([D, D], f32, name="w_f", tag="w_f")
        nc.sync.dma_start(w_f, w_d)
        w_bf = consts.tile([D, D], bf16, name="w_bf", tag="w_bf")
        nc.vector.tensor_copy(w_bf, w_f)

        vd_f = consts.tile([D, 1], f32, name="vd_f", tag="vd_f")
        nc.sync.dma_start(vd_f, vd_d)
        # sliding-window delta weights: zeros with vd at column NPAIR-1
        vdw = consts.tile([D, 2 * NPAIR - 1], bf16, name="vdw", tag="vdw")
        nc.vector.memset(vdw, 0.0)
        nc.vector.tensor_copy(vdw[:, NPAIR - 1 : NPAIR], vd_f)

        # q rows interleaved even/odd: partition t holds [q[2t,:] | q[2t+1,:]]
        q_eo = consts.tile([NPAIR, 2 * D], f32, name="q_eo", tag="q_eo")
        nc.sync.dma_start(q_eo, q_d.rearrange("(t two) d -> t (two d)", two=2))

        # transposes: qT [d, tq] (f32, used as per-partition scalars), pT bf16
        qT = consts.tile([D, TQ], f32, name="qT", tag="qT")
        pT = consts.tile([D, TP], bf16, name="pT", tag="pT")
        for src_d, dstT, nm in ((q_d, qT, "q"), (p_d, pT, "p")):
            tr_ps = s_pool.tile([128, gp * 512], f32, name=f"trps_{nm}", tag="s")
            for h in range(2):
                nat = nat_pool.tile([128, D], f32, name=f"nat_{nm}{h}", tag="nat")
                nc.sync.dma_start(nat, src_d[h * 128 : (h + 1) * 128, :])
                nc.tensor.transpose(tr_ps[:, h * 128 : (h + 1) * 128], nat, ident)
            nc.vector.tensor_copy(dstT[:, 0:128], tr_ps[:, 0:128])
            nc.vector.tensor_copy(dstT[:, 128:256], tr_ps[:, 128:256])

        zeros_sb = consts.tile([128, 2 * TP], bf16, name="zeros_sb", tag="zeros")
        nc.vector.memset(zeros_sb, 0.0)

        # warm the ACT table set (exp_and_others holds both Tanh and Exp) during
        # setup so the ~2.7us PSEUDO_LOAD_ACT_FUNC_SET overlaps the input DMAs
        # instead of stalling the first in-loop tanh
        act_warm = consts.tile([128, 1], f32, name="act_warm", tag="warm")
        nc.vector.memset(act_warm, 0.0)
        nc.scalar.activation(act_warm, act_warm, AF.Tanh)
        nc.scalar.activation(act_warm, act_warm, AF.Exp)

        # ---------------- main compute body ----------------
        # scores^T accumulator: row i = [scores[:,2i].T | scores[:,2i+1].T].
        # Delta matmuls are col-tiled: pair i -> PE col-group i//32, so rows
        # 32g..32g+31 are written by 32-column matmuls at tile_position (0,32g).
        # Pre-clear the bank with 4 zero-matmuls (start=True, stop=False) so
        # every delta accumulates with start=False (no mid-stream bank clears).
        def body():
            sc_ps = sc_pool.tile([128, 2 * TP], f32, name="sc_ps", tag="sc")
            if delta_mode == "col":
                for g in range(4):
                    nc.tensor.matmul(
                        sc_ps[32 * g : 32 * (g + 1), :],
                        lhsT=zeros_sb[:, 0:32],
                        rhs=zeros_sb,
                        start=True,
                        stop=False,
                        skip_group_check=True,
                        tile_position=(0, 32 * g),
                    )
            else:
                nc.tensor.matmul(
                    sc_ps,
                    lhsT=zeros_sb[:, 0:128],
                    rhs=zeros_sb,
                    start=True,
                    stop=False,
                    skip_group_check=True,
                )

            # process pairs in col-group-interleaved order so consecutive delta
            # matmuls target different PE column groups (they run concurrently)
            order = [32 * g + c for c in range(32) for g in range(4)]
            groups = [order[k : k + gp] for k in range(0, len(order), gp)]

            for gi, pairs in enumerate(groups):
                n = len(pairs)
                s_ps = s_pool.tile([128, gp * 512], f32, name=f"s_ps_{gi}", tag="s")
                t_sb = t_pool.tile([128, gp * 512], bf16, name=f"t_sb_{gi}", tag="t")
                for k, i in enumerate(pairs):
                    prod = prod_pool.tile(
                        [128, 512], bf16, name=f"prod_{i}", tag="prod"
                    )
                    nc.vector.tensor_scalar_mul(
                        prod[:, 0:TP], pT, qT[:, 2 * i : 2 * i + 1]
                    )
                    nc.vector.tensor_scalar_mul(
                        prod[:, TP : 2 * TP], pT, qT[:, 2 * i + 1 : 2 * i + 2]
                    )
                    nc.tensor.matmul(
                        s_ps[:, k * 512 : (k + 1) * 512],
                        lhsT=w_bf,
                        rhs=prod,
                        start=True,
                        stop=True,
                    )
                nc.scalar.activation(
                    t_sb[:, : n * 512], s_ps[:, : n * 512], AF.Tanh
                )
                for k, i in enumerate(pairs):
                    if delta_mode == "col":
                        g, c = i // 32, i % 32
                        nc.tensor.matmul(
                            sc_ps[32 * g : 32 * (g + 1), :],
                            lhsT=vdw[:, NPAIR - 1 - c : NPAIR - 1 - c + 32],
                            rhs=t_sb[:, k * 512 : (k + 1) * 512],
                            start=False,
                            stop=(c == 31),
                            skip_group_check=True,
                            tile_position=(0, 32 * g),
                        )
                    else:
                        nc.tensor.matmul(
                            sc_ps,
                            lhsT=vdw[:, NPAIR - 1 - i : 2 * NPAIR - 1 - i],
                            rhs=t_sb[:, k * 512 : (k + 1) * 512],
                            start=False,
                            stop=(i == NPAIR - 1),
                            skip_group_check=True,
                        )

            # ---------- softmax (over tp, the free axis) + output ----------
            e_sb = consts.tile([128, 2 * TP], f32, name="e_sb", tag="e_sb")
            z = consts.tile([128, 2], f32, name="z", tag="z")
            nc.scalar.activation(
                e_sb[:, 0:TP], sc_ps[:, 0:TP], AF.Exp, accum_out=z[:, 0:1]
            )
            nc.scalar.activation(
                e_sb[:, TP : 2 * TP], sc_ps[:, TP : 2 * TP], AF.Exp, accum_out=z[:, 1:2]
            )
            rz = consts.tile([128, 2], f32, name="rz", tag="rz")
            nc.vector.reciprocal(rz, z)
            qn = consts.tile([128, 2 * D], f32, name="qn", tag="qn")
            nc.vector.tensor_scalar_mul(qn[:, 0:D], q_eo[:, 0:D], rz[:, 0:1])
            nc.vector.tensor_scalar_mul(
                qn[:, D : 2 * D], q_eo[:, D : 2 * D], rz[:, 1:2]
            )

            # out[tp,d] = sum_i E_even[i,tp]*qn_even[i,d] + E_odd[i,tp]*qn_odd[i,d]
            out_ps = sc_pool.tile([128, TP], f32, name="out_ps", tag="sc")
            for c in range(2):
                nc.tensor.matmul(
                    out_ps[:, c * D : (c + 1) * D],
                    lhsT=e_sb[:, c * 128 : (c + 1) * 128],
                    rhs=qn[:, 0:D],
                    start=True,
                    stop=False,
                )
                nc.tensor.matmul(
                    out_ps[:, c * D : (c + 1) * D],
                    lhsT=e_sb[:, TP + c * 128 : TP + (c + 1) * 128],
                    rhs=qn[:, D : 2 * D],
                    start=False,
                    stop=True,
                )
            out_sb = consts.tile([128, TP], f32, name="out_sb", tag="out_sb")
            nc.vector.tensor_copy(out_sb, out_ps)
            nc.sync.dma_start(out_d[0:128, :], out_sb[:, 0:128])
            nc.sync.dma_start(out_d[128:256, :], out_sb[:, 128:256])

        if repeat == 1:
            body()
        else:
            # timing-only path: body > 256 instructions/engine, so arm the
            # back-edge branch prefetch to keep the loop overhead small
            with tc.For_i(
                0,
                repeat,
                1,
                hint_engines=(
                    mybir.EngineType.PE,
                    mybir.EngineType.DVE,
                    mybir.EngineType.Activation,
                ),
            ):
                body()

    nc.compile()
    return nc


def get_nc(repeat=1, **kw):
    key = ("nc", repeat, tuple(sorted(kw.items())))
    if key not in _nc_cache:
        _nc_cache[key] = _build_nc(repeat, **kw)
    return _nc_cache[key]


last_results = None


def kernel(q, p, W, vd, _repeat=1, **_kw):
    global last_results
    from concourse.bass_utils import run_bass_kernel_spmd

    q = np.ascontiguousarray(np.asarray(q), dtype=np.float32)
    p = np.ascontiguousarray(np.asarray(p), dtype=np.float32)
    W = np.ascontiguousarray(np.asarray(W), dtype=np.float32)
    vd = np.ascontiguousarray(np.asarray(vd), dtype=np.float32)

    nc = get_nc(_repeat, **_kw)
    in_maps = [
        {"q": q[b], "p": p[b], "W": W, "vd": vd} for b in range(B)
    ]
    res = run_bass_kernel_spmd(nc, in_maps, core_ids=list(range(NCORES)))
    last_results = res
    return np.stack([r["out"] for r in res.results], axis=0)
